# revision 5
# baseline (speedup 1.0000x reference)
"""Trainium2 Bass kernel for nn_MultiHeadAttention (N=2048, D=1024, H=16, causal).

Sharding: 16 heads split across 8 NeuronCores (2 heads/core, tensor-parallel
on the head dim).  Each core projects Q^T/K^T (its 128 head-dims x full
sequence) and V for its heads, computes causal attention in scores-transposed
layout ([nk, nq] blocks, softmax along the nk partition axis), applies its
128-row slice of Wo, and writes an fp16 partial [2048, 1024] output.  The
host sums the 8 partials and adds bo + bv@Wo_slice^T ("all-reduce after W_o"
done host-side; the V bias is mathematically a constant output row, so it
never touches the device).

v2 redesign (75.8us -> target ~48us modeled):
  - PV flipped: probs blocks [128k, 128q] are the *stationary* operand and
    the ones-augmented V [128k, 65] the moving one, so each PV matmul costs
    65 free-columns instead of 128, and the softmax denominator lands on the
    *partition* axis of the PV output.  Normalisation collapses to a
    per-partition reciprocal + scale (no denominator staging row, no PE
    broadcast, no [64,512] reciprocal).
  - The normalized attention output [q, dims] is flipped back to the
    Wo-stationary layout [dims, q] with the DMA XBAR transpose (14ns per
    16x128 tile), not PE/DVE work.
  - Scores computed per 1024-wide q-supertile (T0/T1) so each causally
    trimmed [128, width<=1024] score block takes ONE exp instruction
    (52 exps total vs 80): the ~185ns fixed Act cost per instruction was
    ~15us of the baseline's Act-engine 46us.  T0 is emitted in 512-wide
    halves so the exp train starts as soon as the first q/k column group
    lands (~2.5us).
  - Input DMA order: all q/k fp8 column tiles first (the score pipeline's
    critical path), v f16 tiles deferred behind them; V projection, PV,
    normalize, transpose and Wo ride the emission-interleaved filler queue.
  - Wo staging copies (PSUM f32 -> SBUF f16) are spread DVE/Pool by slack.
"""
import os
import sys

for _p in ("/opt/trn_rl_repo", "/root/.axon_site/_ro/trn_rl_repo"):
    if os.path.isdir(_p) and _p not in sys.path:
        sys.path.append(_p)

import numpy as np

import concourse.bass as bass
import concourse.mybir as mybir
from concourse import bacc
from concourse.bass_utils import run_bass_kernel_spmd
from concourse.tile import TileContext
from contextlib import ExitStack

N = 2048
D = 1024
NCORES = 8
DL = 128

F32 = mybir.dt.float32
F16 = mybir.dt.float16
F8 = mybir.dt.float8e4

# fp8 Q/K path: q, k, Wq, Wk in e4m3 (weights host-scaled x16 to clear the
# e4m3 denormal floor; bq, bk scaled to match; the extra 16*16 factor on the
# scores folds into the exp scale).  V path, probs, attnT, Wo stay f16 --
# every attempted fp8 extension of those paths measured >2e-2 end-to-end.
EXP_SCALE = 0.125 / 256.0


def build_nc(opts=None):
    nc = bacc.Bacc("TRN2", target_bir_lowering=False, debug=False,
                   num_devices=NCORES)

    qP = nc.dram_tensor("qP", [128, 8, N], F8, kind="ExternalInput")
    kP = nc.dram_tensor("kP", [128, 8, N], F8, kind="ExternalInput")
    vP = nc.dram_tensor("vP", [128, 8, N], F16, kind="ExternalInput")
    wqP = nc.dram_tensor("wqP", [128, 8, DL], F8, kind="ExternalInput")
    wkP = nc.dram_tensor("wkP", [128, 8, DL], F8, kind="ExternalInput")
    wvP = nc.dram_tensor("wvP", [128, 8, DL], F16, kind="ExternalInput")
    woP = nc.dram_tensor("woP", [DL, D], F16, kind="ExternalInput")
    bqk = nc.dram_tensor("bqk", [DL, 2], F32, kind="ExternalInput")
    out = nc.dram_tensor("out", [N, D], F16, kind="ExternalOutput")

    AF = mybir.ActivationFunctionType
    ALU = mybir.AluOpType
    DR = mybir.MatmulPerfMode.DoubleRow

    with TileContext(nc) as tc, ExitStack() as ctx:
        const = ctx.enter_context(tc.tile_pool(name="const", bufs=1))
        big = ctx.enter_context(tc.tile_pool(name="big", bufs=1))
        colp = ctx.enter_context(tc.tile_pool(name="colp", bufs=1))
        probsp = ctx.enter_context(tc.tile_pool(name="probsp", bufs=1))
        rcqp = ctx.enter_context(tc.tile_pool(name="rcqp", bufs=2))
        outp = ctx.enter_context(tc.tile_pool(name="outp", bufs=3))

        # ---- input streaming, one sync-queue in arrival-priority order:
        # ALL q/k fp8 tiles first (they feed the exp train, the kernel's
        # pacer), then wv/wo, then the v f16 tiles (V-proj/PV/Wo ride the
        # filler queue and only unblock from ~17us on) ----
        wq = const.tile([128, 8, DL], F8)
        nc.sync.dma_start(wq[:], wqP[:])
        wk = const.tile([128, 8, DL], F8)
        nc.sync.dma_start(wk[:], wkP[:])
        bqk_t = const.tile([DL, 2], F32)
        nc.sync.dma_start(bqk_t[:], bqk[:])
        qc, kc, vc = [], [], []

        def load_col(lst, name, dram, c, dt):
            t = colp.tile([128, 8, 512], dt, name=f"{name}{c}")
            nc.sync.dma_start(t[:], dram[:, :, 512 * c:512 * (c + 1)])
            lst.append(t)

        for c in range(4):
            load_col(qc, "qc", qP, c, F8)
            load_col(kc, "kc", kP, c, F8)
        wv = const.tile([128, 8, DL], F16)
        nc.sync.dma_start(wv[:], wvP[:])
        wo = const.tile([DL, D], F16)
        nc.sync.dma_start(wo[:], woP[:])
        for c in range(4):
            load_col(vc, "vc", vP, c, F16)

        ones64 = const.tile([1, 64], F16)
        nc.vector.memset(ones64[:], 1.0)
        ones512 = const.tile([1, 512], F16)
        nc.vector.memset(ones512[:], 1.0)

        # ---- persistent activations ----
        QT = big.tile([128, N], F16)
        KT = big.tile([128, N], F16)
        # Vaug[p_seq, head, seq_block, 65]: cols 0:64 projected V, col 64
        # ones (PV's moving operand; the ones column accumulates the softmax
        # denominator into PV-output column 64 for free).
        Vaug = big.tile([128, 2, 16, 65], F16)
        nc.vector.memset(Vaug[:, :, :, 64:65], 1.0)
        attnT = big.tile([128, N], F16)
        attnQ = [big.tile([128, 8, 128], F16, name=f"attnQ{T}")
                 for T in range(2)]

        # probs segments: (T, h, b) -> list of (qlo, qhi, pb_tile)
        probs_seg = {}

        with tc.tile_pool(name="scp", bufs=2, space="PSUM") as scp, \
             tc.tile_pool(name="wps", bufs=1, space="PSUM") as wps, \
             tc.tile_pool(name="pvqp", bufs=2, space="PSUM") as pvqp:

            # The PE SEQ is in-order and sem-waits block it, so emission
            # interleaves independent "filler" work (V projection, PV
            # strip sweeps, normalize, transpose, Wo) between score blocks,
            # paced against the Activation-engine exp train.
            fillers = []

            def emit_score(T, h, b, qlo, qhi):
                # one causally-trimmed score block: k in [128b, 128b+128),
                # q in [qlo, qhi).  Matmuls split at PSUM bank boundaries;
                # ONE exp covers the whole [128, qhi-qlo] region.
                w = qhi - qlo
                r0, r1 = 64 * h, 64 * (h + 1)
                sc = scp.tile([128, 2, 512], F32, name="sc")
                flat = sc.rearrange("p a b -> p (a b)")
                for c0 in range(0, w, 512):
                    c1 = min(c0 + 512, w)
                    nc.tensor.matmul(
                        flat[:, c0:c1],
                        KT[r0:r1, 128 * b:128 * (b + 1)],
                        QT[r0:r1, qlo + c0:qlo + c1],
                        start=True, stop=True, tile_position=(64 * h, 0))
                pb = probsp.tile([128, w], F16, name=f"pb{T}{h}{b}_{qlo}")
                nc.scalar.activation(pb[:], flat[:, 0:w], AF.Exp,
                                     scale=EXP_SCALE)
                if 128 * b >= qlo:
                    # diagonal chunk starts this segment: causal-mask it
                    nc.gpsimd.affine_select(
                        out=pb[:, 0:128], in_=pb[:, 0:128],
                        compare_op=ALU.is_ge, fill=0.0,
                        base=0, pattern=[[1, 128]], channel_multiplier=-1)
                probs_seg.setdefault((T, h, b), []).append((qlo, qhi, pb))

            def probs_slice(T, h, b, s):
                q0 = 1024 * T + 128 * s
                for qlo, qhi, pb in probs_seg[(T, h, b)]:
                    if qlo <= q0 < qhi:
                        return pb[:, q0 - qlo:q0 - qlo + 128]
                raise AssertionError((T, h, b, s))

            def emit_vproj(c, j):
                # seq block blk = 4c+j -> vp region [:, 128j:128j+128]
                blk = 4 * c + j
                if j == 0:
                    emit_vproj.vp = scp.tile([128, 2, 512], F32, name="sc")
                vp = emit_vproj.vp.rearrange("p a b -> p (a b)")
                for u in range(8):
                    nc.tensor.matmul(
                        vp[:, 128 * j:128 * (j + 1)],
                        vc[c][:, u, 128 * j:128 * (j + 1)],
                        wv[:, u, :], start=(u == 0), stop=(u == 7))

            def emit_vcopy(c):
                vp = emit_vproj.vp.rearrange(
                    "p a (bb g d) -> p a g bb d", bb=4, g=2, d=64)
                nc.vector.tensor_scalar_mul(
                    Vaug[:, 0:2, 4 * c:4 * (c + 1), 0:64],
                    vp[:, 0, :, :, :], 1.0)

            pvq_ref = {}

            def emit_pv(T, h, s):
                # strip s of supertile T: accumulate over all k blocks
                # b = 0..8T+s into pvq[128q, 65] (probs stationary, Vaug
                # moving: 65 free-cols per matmul; col 64 = denominator).
                sg, si = s // 4, s % 4
                if si == 0:
                    pvq_ref[(T, h, sg)] = pvqp.tile([128, 4, 65], F32,
                                                    name="pvq")
                pvq = pvq_ref[(T, h, sg)]
                last = 8 * T + s
                for b in range(last + 1):
                    nc.tensor.matmul(
                        pvq[:, si, 0:65],
                        probs_slice(T, h, b, s),
                        Vaug[:, h, b, 0:65],
                        start=(b == 0), stop=(b == last))

            def emit_norm(T, h, sg):
                pvq = pvq_ref[(T, h, sg)]
                rcq = rcqp.tile([128, 4, 1], F32, name="rcq")
                nc.vector.reciprocal(rcq[:], pvq[:, :, 64:65])
                nc.vector.tensor_mul(
                    attnQ[T][:, 4 * sg:4 * (sg + 1), 64 * h:64 * (h + 1)],
                    pvq[:, :, 0:64],
                    rcq.broadcast_to((128, 4, 64)))

            def emit_norm_strip(T, h, s):
                # per-strip normalize for the final strip-group, so the
                # tail pipelines strip-by-strip instead of per-quad
                sg, si = s // 4, s % 4
                pvq = pvq_ref[(T, h, sg)]
                rcq = rcqp.tile([128, 4, 1], F32, name="rcq")
                nc.vector.reciprocal(rcq[:, 0:1, :], pvq[:, si:si + 1, 64:65])
                nc.vector.tensor_mul(
                    attnQ[T][:, s:s + 1, 64 * h:64 * (h + 1)],
                    pvq[:, si:si + 1, 0:64],
                    rcq[:, 0:1, :].broadcast_to((128, 1, 64)))

            def emit_transpose(T, s):
                m = 8 * T + s
                nc.sync.dma_start_transpose(
                    attnT[:, 128 * m:128 * (m + 1)], attnQ[T][:, s, :])

            def emit_wo(T, s, stage):
                m = 8 * T + s
                wp = wps.tile([128, 2, 512], F32, name="wp")
                for u in range(2):
                    nc.tensor.matmul(wp[:, u, :],
                                     attnT[:, 128 * m:128 * (m + 1)],
                                     wo[:, 512 * u:512 * (u + 1)],
                                     start=True, stop=True)
                ob = outp.tile([128, 1024], F16, name="ob")
                src = wp.rearrange("p a b -> p (a b)")
                if stage == "act":
                    nc.scalar.activation(ob[:], src[:], AF.Copy)
                elif stage == "pool":
                    nc.gpsimd.tensor_scalar_mul(ob[:], src[:], 1.0)
                else:
                    nc.vector.tensor_scalar_mul(ob[:], src[:], 1.0)
                nc.sync.dma_start(out[128 * m:128 * (m + 1), :], ob[:])

            def emit_proj(g):
                # QT/KT columns [512g : 512g+512] from qc[g]/kc[g], fp8
                # DoubleRow (256-wide contraction per pass)
                ps = scp.tile([128, 2, 512], F32, name="sc")
                for src_c, w, half in ((qc[g], wq, 0), (kc[g], wk, 1)):
                    for jj in range(4):
                        nc.tensor.matmul(
                            ps[:, half, :], w[:, 2 * jj:2 * jj + 2, :],
                            src_c[:, 2 * jj:2 * jj + 2, :],
                            start=(jj == 0), stop=(jj == 3),
                            perf_mode=DR)
                nc.vector.tensor_scalar_add(
                    QT[:, 512 * g:512 * (g + 1)], ps[:, 0, :],
                    bqk_t[:, 0:1])
                nc.vector.tensor_scalar_add(
                    KT[:, 512 * g:512 * (g + 1)], ps[:, 1, :],
                    bqk_t[:, 1:2])

            def warmup(n=1):
                # keep the tensor engine busy through the DMA front so the
                # p-state ramp completes before the first projection
                for _ in range(n):
                    wu = wps.tile([128, 2, 512], F32, name="wp")
                    nc.tensor.matmul(wu[0:64, 0, :], ones64[:], ones512[:],
                                     start=True, stop=True)

            import math

            blocks_left = [32]

            def pop_fillers():
                k = max(2, min(5, math.ceil(
                    len(fillers) / max(1, blocks_left[0]))))
                for _ in range(k):
                    if fillers:
                        fillers.pop(0)()

            def queue_vproj(c):
                fillers.extend(lambda c=c, j=j: emit_vproj(c, j)
                               for j in range(4))
                fillers.append(lambda c=c: emit_vcopy(c))

            def queue_pv(T, h, sg):
                for si in range(4):
                    fillers.append(lambda T=T, h=h, s=4 * sg + si:
                                   emit_pv(T, h, s))
                fillers.append(lambda T=T, h=h, sg=sg:
                               emit_norm(T, h, sg))

            # transposes+Wo for strip-group sg of supertile T (after the
            # second head's norm); staging engine: T0 -> DVE (slack early),
            # T1 -> Pool, with the final strips split across Pool/DVE/Act
            # (Act idles once the exp train drains).
            def queue_tail(T, sg):
                for si in range(4):
                    fillers.append(lambda T=T, s=4 * sg + si:
                                   emit_transpose(T, s))
                for si in range(4):
                    s = 4 * sg + si
                    stage = "dve" if T == 0 else "pool"
                    fillers.append(lambda T=T, s=s, st=stage:
                                   emit_wo(T, s, st))

            # ---------------- emission schedule ----------------
            warmup(4)
            emit_proj(0)
            warmup(2)

            # T0, both heads in 512-wide halves (half A needs only qc0/kc0,
            # so the exp train starts ~2.5us in; half B follows proj(1))
            for h in range(2):
                for b in range(4):
                    emit_score(0, h, b, 128 * b, 512)
                if h == 0:
                    emit_proj(1)
            for h in range(2):
                for b in range(8):
                    emit_score(0, h, b, max(512, 128 * b), 1024)
            # T0 pipeline, producer-before-consumer in FIFO order:
            # V group c covers Vaug blocks 4c..4c+3, needed by strip-group
            # sg=c of T0 (strips s sweep k-blocks 0..s).
            queue_vproj(0)
            queue_pv(0, 0, 0)
            queue_pv(0, 1, 0)
            queue_tail(0, 0)
            queue_vproj(1)
            queue_pv(0, 0, 1)
            queue_pv(0, 1, 1)
            queue_tail(0, 1)

            emit_proj(2)
            emit_proj(3)
            for h in range(2):
                for b in range(16):
                    emit_score(1, h, b, max(1024, 128 * b), 2048)
                    blocks_left[0] -= 1
                    pop_fillers()
                    if h == 0 and b == 2:
                        queue_vproj(2)
                    if h == 0 and b == 6:
                        queue_vproj(3)
                    if b == 11:
                        # pb for k-blocks 0..11 emitted: strip-group 0
                        # (strips 0..3 sweep b<=8+s<=11) is ready
                        queue_pv(1, h, 0)
                        if h == 1:
                            queue_tail(1, 0)
                if h == 0:
                    queue_pv(1, 0, 1)
            # final strip-group (T1, h1, sg1): per-strip pipelining so the
            # tail chain after the last exp is one strip deep, not four
            for si in range(4):
                s = 4 + si
                fillers.append(lambda s=s: emit_pv(1, 1, s))
                fillers.append(lambda s=s: emit_norm_strip(1, 1, s))
                fillers.append(lambda s=s: emit_transpose(1, s))
                stage = ("pool", "dve", "act", "act")[si]
                fillers.append(lambda s=s, st=stage: emit_wo(1, s, st))
            while fillers:
                fillers.pop(0)()

    nc.compile()
    return nc


def make_in_maps(q, k, v, Wq, bq, Wk, bk, Wv, bv, Wo, bo):
    import ml_dtypes
    fp8 = ml_dtypes.float8_e4m3
    f32 = np.float32
    WSCALE = 16.0

    def pack_cols(x, dt):
        # [N, D] input -> x.T [D, N] -> [128, 8, N] with row (j*128+p) at
        # [p, j, :]
        xt = np.ascontiguousarray(x.T.astype(f32))
        return np.ascontiguousarray(
            xt.reshape(8, 128, N).transpose(1, 0, 2)).astype(dt)

    qPa, kPa = pack_cols(q, fp8), pack_cols(k, fp8)
    vPa = pack_cols(v, np.float16)
    WqT = Wq.T.astype(f32) * WSCALE
    WkT = Wk.T.astype(f32) * WSCALE
    WvT = Wv.T.astype(f32)
    WoT = Wo.T.astype(f32)

    def pack_w(WT, c, dt):
        # [D, DL] column slice -> [128, 8, DL]
        sl = np.ascontiguousarray(WT[:, DL * c:DL * (c + 1)])
        return np.ascontiguousarray(
            sl.reshape(8, 128, DL).transpose(1, 0, 2)).astype(dt)

    in_maps = []
    for c in range(NCORES):
        d0 = DL * c
        in_maps.append({
            "qP": qPa, "kP": kPa, "vP": vPa,
            "wqP": pack_w(WqT, c, fp8),
            "wkP": pack_w(WkT, c, fp8),
            "wvP": pack_w(WvT, c, np.float16),
            "woP": np.ascontiguousarray(WoT[d0:d0 + DL, :]).astype(np.float16),
            "bqk": np.ascontiguousarray(
                np.stack([bq[d0:d0 + DL] * WSCALE,
                          bk[d0:d0 + DL] * WSCALE], axis=1)).astype(f32),
        })
    return in_maps


_NC_CACHE = None


def _get_nc():
    global _NC_CACHE
    if _NC_CACHE is None:
        _NC_CACHE = build_nc()
    return _NC_CACHE


def kernel(q, k, v, Wq, bq, Wk, bk, Wv, bv, Wo, bo):
    """Full-input / full-output entry point (harness contract)."""
    q, k, v = np.asarray(q), np.asarray(k), np.asarray(v)
    Wq, bq, Wk, bk = np.asarray(Wq), np.asarray(bq), np.asarray(Wk), np.asarray(bk)
    Wv, bv, Wo, bo = np.asarray(Wv), np.asarray(bv), np.asarray(Wo), np.asarray(bo)
    nc = _get_nc()
    in_maps = make_in_maps(q, k, v, Wq, bq, Wk, bk, Wv, bv, Wo, bo)
    res = run_bass_kernel_spmd(nc, in_maps, list(range(NCORES)))
    acc = res.results[0]["out"].astype(np.float64)
    for c in range(1, NCORES):
        acc += res.results[c]["out"]
    # V bias folded host-side: concat rows carry +bv per head-dim, so the
    # device-side output is short exactly bv @ Wo^T (a constant row).
    acc += (bv.astype(np.float64) @ Wo.T.astype(np.float64))
    acc += bo.astype(np.float64)
    return acc.astype(np.float32)
